# revision 1
# baseline (speedup 1.0000x reference)
"""Trainium2 Bass kernel for nn_LocalGeometryLoss.

Reference semantics (fp32):
    hp = l2norm(hidden_previous)                    # [8192, 768]
    sim = hp @ hp.T                                 # cosine similarity
    nbr = top_k(sim, 6)[:, 1:]                      # 5 nearest neighbors (self dropped)
    e[i,k] = +1 if labels_prev[i]==labels_prev[nbr[i,k]] else -1
    hc = l2norm(hidden_current)                     # [4096, 768]
    d2[i,j] = max(|hc_i|^2 + |hc_j|^2 - 2 hc_i.hc_j, 0)
    loss = 0.5 * sum_{i<4096, nbr j<4096} e * d2[i, nbr] / 4096^2

Only rows i < 4096 and neighbor columns j < 4096 contribute, so each of the
8 cores handles 512 of the 4096 contributing rows: it computes its
[512, 8192] block of the similarity matrix (bf16 matmul, fp32 accumulate),
extracts top-8 per row with the DVE max/max_index ops, gathers neighbor
rows/labels with indirect DMA, and reduces its partial loss to a scalar.
The host sums the 8 partial scalars.

Row-normalizing the lhs is skipped: a positive per-row scale does not
change that row's top-k ordering, and the self column still dominates.
"""

import numpy as np

import concourse.bass as bass
import concourse.bacc as bacc
import concourse.mybir as mybir
from concourse import tile
from concourse.bass_utils import run_bass_kernel_spmd

FP = mybir.dt.float32
BF = mybir.dt.bfloat16
U32 = mybir.dt.uint32

B_PREV = 8192
B_CURR = 4096
D = 768
KNBR = 5
WEIGHT = 0.5
N_CORES = 8
ROWS_PER_CORE = B_CURR // N_CORES          # 512
M_TILES = ROWS_PER_CORE // 128             # 4
KC = D // 128                              # 6 contraction chunks
NC_CHUNK = 512
N_CHUNKS = B_PREV // NC_CHUNK              # 16
TBL_W = 772                                # 768 hc + 1 label + 3 pad
ACT = mybir.ActivationFunctionType
ALU = mybir.AluOpType

_CACHE = {}


def _build():
    nc = bacc.Bacc("TRN2", target_bir_lowering=False, debug=False,
                   num_devices=N_CORES, num_swdge_queues=4)

    hp_full = nc.dram_tensor("hp_full", [B_PREV, D], FP, kind="ExternalInput").ap()
    hp_own = nc.dram_tensor("hp_own", [ROWS_PER_CORE, D], FP, kind="ExternalInput").ap()
    hc_own = nc.dram_tensor("hc_own", [ROWS_PER_CORE, D], FP, kind="ExternalInput").ap()
    # [4096, 772] fp32: cols 0:768 raw hidden_current row j, col 768 labels_prev[j]
    tbl = nc.dram_tensor("tbl", [B_CURR, TBL_W], FP, kind="ExternalInput").ap()
    # labels of own 512 prev rows, [4, 128]
    lbl_own = nc.dram_tensor("lbl_own", [M_TILES, 128], FP, kind="ExternalInput").ap()

    partial = nc.dram_tensor("partial", [1, 1], FP, kind="ExternalOutput").ap()
    idx_dbg = nc.dram_tensor("idx_dbg", [M_TILES, 128, 8], U32, kind="ExternalOutput").ap()

    with tile.TileContext(nc) as tc:
        sb = tc.alloc_tile_pool(name="sb", bufs=1)
        stage = tc.alloc_tile_pool(name="stage", bufs=2)
        small = tc.alloc_tile_pool(name="small", bufs=2)
        scratch = tc.alloc_tile_pool(name="scratch", bufs=2)
        simp = tc.alloc_tile_pool(name="simp", bufs=2)
        psp = tc.alloc_tile_pool(name="psp", bufs=6, space="PSUM")
        psp1 = tc.alloc_tile_pool(name="psp1", bufs=1, space="PSUM")

        # persistent tiles
        hpT = sb.tile([128, KC, B_PREV], BF)          # rhs: normalized hp, transposed
        lhsT = sb.tile([128, KC, ROWS_PER_CORE], BF)  # own rows, transposed (unnormalized)
        s_bf = sb.tile([128, M_TILES, D], BF)         # own hc rows, normalized
        lbl_sb = sb.tile([128, M_TILES], FP)          # own labels
        acc = sb.tile([128, M_TILES * KNBR], FP)      # per-row loss terms
        ones = sb.tile([128, 1], FP)
        twos = sb.tile([128, 1], FP)

        nc.vector.memset(ones[:], 1.0)
        nc.vector.memset(twos[:], 2.0)
        nc.sync.dma_start(lbl_sb[:], lbl_own.rearrange("m p -> p m"))

        # ---- own rows -> lhsT (bf16 cast + transpose; no normalization) ----
        for i in range(M_TILES):
            t = stage.tile([128, D], FP, tag="hpstage")
            nc.sync.dma_start(t[:], hp_own[128 * i:128 * (i + 1), :])
            tb = stage.tile([128, D], BF, tag="ownbf")
            nc.vector.tensor_copy(tb[:], t[:])
            nc.sync.dma_start_transpose(lhsT[:, :, 128 * i:128 * (i + 1)], tb[:])

        # ---- own hc rows -> s_bf (normalized) ----
        for i in range(M_TILES):
            t = stage.tile([128, D], FP, tag="hpstage")
            nc.sync.dma_start(t[:], hc_own[128 * i:128 * (i + 1), :])
            sq = scratch.tile([128, D], BF, tag="sq")
            ss = small.tile([128, 1], FP, tag="ss")
            nc.scalar.activation(sq[:], t[:], ACT.Square, accum_out=ss[:])
            rt = small.tile([128, 1], FP, tag="rt")
            nc.scalar.sqrt(rt[:], ss[:])
            inv = small.tile([128, 1], FP, tag="inv")
            nc.vector.reciprocal(inv[:], rt[:])
            nc.vector.tensor_scalar(out=s_bf[:, i, :], in0=t[:], scalar1=inv[:, :1],
                                    scalar2=None, op0=ALU.mult)

        # ---- full hp: normalize -> bf16 -> transpose into hpT ----
        # process 64 chunks of 128 rows; 2 chunks per DMA
        for ci in range(64):
            t2 = stage.tile([128, D], FP, tag="hpstage")
            nc.sync.dma_start(t2[:], hp_full[128 * ci:128 * (ci + 1), :])
            sq = scratch.tile([128, D], BF, tag="sq")
            ss = small.tile([128, 1], FP, tag="ss")
            nc.scalar.activation(sq[:], t2[:], ACT.Square, accum_out=ss[:])
            rt = small.tile([128, 1], FP, tag="rt")
            nc.scalar.sqrt(rt[:], ss[:])
            inv = small.tile([128, 1], FP, tag="inv")
            nc.vector.reciprocal(inv[:], rt[:])
            hb = stage.tile([128, D], BF, tag="hpbf")
            nc.vector.tensor_scalar(out=hb[:], in0=t2[:], scalar1=inv[:, :1],
                                    scalar2=None, op0=ALU.mult)
            nc.scalar.dma_start_transpose(hpT[:, :, 128 * ci:128 * (ci + 1)], hb[:])

        # ---- per m-tile: gram row block, topk, gather, partial terms ----
        for m in range(M_TILES):
            sim = simp.tile([128, B_PREV], FP, tag="sim")
            for n in range(N_CHUNKS):
                ps = psp.tile([128, NC_CHUNK], FP, tag="ps")
                for k in range(KC):
                    nc.tensor.matmul(
                        ps[:],
                        lhsT[:, k, 128 * m:128 * (m + 1)],
                        hpT[:, k, NC_CHUNK * n:NC_CHUNK * (n + 1)],
                        start=(k == 0), stop=(k == KC - 1))
                nc.scalar.copy(sim[:, NC_CHUNK * n:NC_CHUNK * (n + 1)], ps[:])

            v8 = small.tile([128, 8], FP, tag="v8")
            i8 = small.tile([128, 8], U32, tag="i8")
            nc.vector.max(out=v8[:], in_=sim[:])
            nc.vector.max_index(out=i8[:], in_max=v8[:], in_values=sim[:])
            nc.sync.dma_start(idx_dbg[m], i8[:])

            # neighbor slots 1..5; clamp indices to <4096 for the gather
            jc = small.tile([128, KNBR], U32, tag="jc")
            nc.vector.tensor_scalar(out=jc[:], in0=i8[:, 1:6], scalar1=B_CURR - 1,
                                    scalar2=None, op0=ALU.min)
            msk = small.tile([128, KNBR], FP, tag="msk")
            nc.vector.tensor_scalar(out=msk[:], in0=i8[:, 1:6], scalar1=B_CURR,
                                    scalar2=None, op0=ALU.is_lt)

            dots = small.tile([128, KNBR], FP, tag="dots")
            ssg = small.tile([128, KNBR], FP, tag="ssg")
            lblg = small.tile([128, KNBR], FP, tag="lblg")
            for s in range(KNBR):
                g = scratch.tile([128, TBL_W], FP, tag="gath")
                nc.gpsimd.indirect_dma_start(
                    out=g[:], out_offset=None, in_=tbl[:],
                    in_offset=bass.IndirectOffsetOnAxis(ap=jc[:, s:s + 1], axis=0))
                nc.vector.tensor_copy(lblg[:, s:s + 1], g[:, D:D + 1])
                # sum of squares of the raw gathered row
                sq = scratch.tile([128, D], BF, tag="sq")
                nc.scalar.activation(sq[:], g[:, :D], ACT.Square, accum_out=ssg[:, s:s + 1])
                # dot with own normalized hc row
                prod = scratch.tile([128, D], BF, tag="prod")
                nc.gpsimd.tensor_tensor(out=prod[:], in0=g[:, :D], in1=s_bf[:, m, :],
                                        op=ALU.mult)
                nc.vector.tensor_reduce(out=dots[:, s:s + 1], in_=prod[:],
                                        axis=mybir.AxisListType.X, op=ALU.add)

            # cos = dot / sqrt(ssg);  d2 = relu(2 - 2 cos)
            rt5 = small.tile([128, KNBR], FP, tag="rt5")
            nc.scalar.sqrt(rt5[:], ssg[:])
            inv5 = small.tile([128, KNBR], FP, tag="inv5")
            nc.vector.reciprocal(inv5[:], rt5[:])
            cos = small.tile([128, KNBR], FP, tag="cos")
            nc.vector.tensor_tensor(out=cos[:], in0=dots[:], in1=inv5[:], op=ALU.mult)
            d2 = small.tile([128, KNBR], FP, tag="d2")
            nc.scalar.activation(d2[:], cos[:], ACT.Relu, bias=twos[:, :1], scale=-2.0)

            # e = 2*(lblg == lbl_own) - 1, masked
            eqv = small.tile([128, KNBR], FP, tag="eqv")
            nc.vector.tensor_scalar(out=eqv[:], in0=lblg[:], scalar1=lbl_sb[:, m:m + 1],
                                    scalar2=None, op0=ALU.is_equal)
            e5 = small.tile([128, KNBR], FP, tag="e5")
            nc.vector.tensor_scalar(out=e5[:], in0=eqv[:], scalar1=2.0, scalar2=-1.0,
                                    op0=ALU.mult, op1=ALU.add)
            em = small.tile([128, KNBR], FP, tag="em")
            nc.vector.tensor_tensor(out=em[:], in0=e5[:], in1=msk[:], op=ALU.mult)
            nc.vector.tensor_tensor(out=acc[:, KNBR * m:KNBR * (m + 1)], in0=em[:],
                                    in1=d2[:], op=ALU.mult)

        # ---- final reduction: acc [128, 20] -> scalar ----
        rowsum = small.tile([128, 1], FP, tag="rowsum")
        nc.vector.tensor_reduce(out=rowsum[:], in_=acc[:], axis=mybir.AxisListType.X,
                                op=ALU.add)
        pps = psp1.tile([1, 1], FP, tag="pps")
        nc.tensor.matmul(pps[:], ones[:], rowsum[:], start=True, stop=True)
        res = small.tile([1, 1], FP, tag="res")
        nc.scalar.copy(res[:], pps[:])
        sc = small.tile([1, 1], FP, tag="sc")
        nc.vector.tensor_scalar_mul(sc[:], res[:], WEIGHT / (B_CURR * B_CURR))
        nc.sync.dma_start(partial[:], sc[:])

        for p in (psp1, psp, simp, scratch, small, stage, sb):
            p.release()

    nc.compile()
    return nc


def _get_nc():
    if "nc" not in _CACHE:
        _CACHE["nc"] = _build()
    return _CACHE["nc"]


def kernel(hidden_current, hidden_previous, labels_current, labels_previous,
           _want_debug=False):
    hp = np.ascontiguousarray(np.asarray(hidden_previous, dtype=np.float32))
    hc = np.ascontiguousarray(np.asarray(hidden_current, dtype=np.float32))
    lp = np.asarray(labels_previous).astype(np.float32)

    tbl = np.empty((B_CURR, TBL_W), dtype=np.float32)
    tbl[:, :D] = hc
    tbl[:, D] = lp[:B_CURR]
    tbl[:, D + 1:] = 0.0

    nc = _get_nc()
    in_maps = []
    for c in range(N_CORES):
        r0 = c * ROWS_PER_CORE
        in_maps.append({
            "hp_full": hp,
            "hp_own": hp[r0:r0 + ROWS_PER_CORE],
            "hc_own": hc[r0:r0 + ROWS_PER_CORE],
            "tbl": tbl,
            "lbl_own": lp[r0:r0 + ROWS_PER_CORE].reshape(M_TILES, 128),
        })
    out = run_bass_kernel_spmd(nc, in_maps, list(range(N_CORES)))
    total = np.float32(0.0)
    for c in range(N_CORES):
        total += out.results[c]["partial"][0, 0]
    result = np.asarray(total, dtype=np.float32)
    if _want_debug:
        return result, out
    return result

